# revision 2
# baseline (speedup 1.0000x reference)
"""RNN-T Joint network kernel for Trainium2 (Bass/Tile), 8-core SPMD.

Problem: out[b,t,u,v] = tanh(enc[b,t,:] + pred[b,u,:]) @ W[v,:] + bias[v]
  B=4, T=256, U=64, D=640, V=4096  (fp32 in/out)

Sharding: data-parallel over (B,T). Core i handles b = i//2, t in
[(i%2)*128, (i%2)*128+128). Each core computes an [128*64, 4096] slice of
the output; no collectives needed.

Device kernel (per core):
  - host pre-transposes operands so the contraction dim D sits on SBUF
    partitions: encT [D,128], predT [D,64], wT [D,V].
  - hT[d, (t,u)] = tanh(predT[d,u] + encT[d,t]) via scalar-engine
    activation with per-partition bias (one instr per (d-chunk, t)).
  - PE matmul per (m-chunk, psum-bank): psum[128m, 512v] +=
    hT[k][:,m].T @ w[k,n] over 5 k-chunks; operands in fp16 so the PE
    streams 1 row/cycle and LDWEIGHTS overlaps the previous matmul
    (f32r serializes the weight load: 272 ns vs 216 ns per matmul).
  - per-bank epilogue: DVE adds bias PSUM->SBUF, then a 256KB DMA out.
    n-outer/k-inner keeps drains and output DMA evenly spread, which
    shrinks the kernel tail to one bank's worth of work.
  - input DMAs are issued in first-consumption order (enc, pred, then
    weight slices in (n,k) order, bias last) so the first matmul isn't
    gated on the full 4MB weight preload.
"""

import os
import sys

import numpy as np

if "/root/.axon_site/_ro/trn_rl_repo" not in sys.path:
    sys.path.append("/root/.axon_site/_ro/trn_rl_repo")

import concourse.bass as bass  # noqa: E402
import concourse.mybir as mybir  # noqa: E402
import concourse.tile as tile  # noqa: E402
from concourse import bacc  # noqa: E402
from concourse.bass_utils import run_bass_kernel_spmd  # noqa: E402

B, T, U, D, V = 4, 256, 64, 640, 4096
N_CORES = 8
T_PER_CORE = T // (N_CORES // B)  # 128
ROWS = T_PER_CORE * U  # 8192 rows per core
KC = D // 128  # 5 k-chunks
NB = V // 512  # 8 psum banks per row-chunk
M_CHUNKS = ROWS // 128  # 64  (each = 2 t values x 64 u)
T_PER_M = 128 // U  # 2

# matmul dtype mode: "fp16"/"bf16" (1 cyc/row, hidden weight loads),
# "f32r" (1 cyc/row but serialized ldweights), "f32" (exact, 4 cyc/row)
MM_MODE = os.environ.get("JOINT_MM_MODE", "fp16")


def build_nc(mode: str):
    nc = bacc.Bacc("TRN2", target_bir_lowering=False, debug=False)

    f32 = mybir.dt.float32
    w_dt = {
        "bf16": mybir.dt.bfloat16,
        "fp16": mybir.dt.float16,
        "f32r": mybir.dt.float32r,
    }.get(mode, f32)

    encT_d = nc.dram_tensor("encT", [D, T_PER_CORE], f32, kind="ExternalInput")
    predT_d = nc.dram_tensor("predT", [D, U], f32, kind="ExternalInput")
    wT_d = nc.dram_tensor("wT", [D, V], w_dt, kind="ExternalInput")
    bias_d = nc.dram_tensor("bias", [1, V], f32, kind="ExternalInput")
    out_d = nc.dram_tensor("out", [ROWS, V], f32, kind="ExternalOutput")

    encT = encT_d.ap().rearrange("(k p) t -> p k t", p=128)
    predT = predT_d.ap().rearrange("(k p) u -> p k u", p=128)
    wT = wT_d.ap().rearrange("(k p) v -> p k v", p=128)
    out = out_d.ap()

    h_dt = w_dt

    with tile.TileContext(nc) as tc:
        with (
            tc.tile_pool(name="singles", bufs=1) as singles,
            tc.tile_pool(name="hpool", bufs=3) as hpool,
            tc.tile_pool(name="opool", bufs=12) as opool,
            tc.tile_pool(name="psum", bufs=8, space="PSUM") as psum_pool,
        ):
            # inputs in first-consumption order: the first matmul needs
            # enc/pred (for tanh) and only the (k=0, n=0) weight slice.
            enc_s = singles.tile([128, KC, T_PER_CORE], f32, tag="enc")
            nc.sync.dma_start(out=enc_s, in_=encT)
            pred_s = singles.tile([128, KC, U], f32, tag="pred")
            nc.sync.dma_start(out=pred_s, in_=predT)
            w_s = {}
            for n in range(NB):
                for k in range(KC):
                    w_s[k, n] = singles.tile(
                        [128, 512], w_dt, tag=f"w{k}_{n}", name=f"w{k}_{n}"
                    )
                    nc.sync.dma_start(
                        out=w_s[k, n], in_=wT[:, k, n * 512 : (n + 1) * 512]
                    )
            bias_s = singles.tile([128, V], f32, tag="bias")
            nc.sync.dma_start(out=bias_s, in_=bias_d.ap().to_broadcast((128, V)))

            for m in range(M_CHUNKS):
                hT = hpool.tile([128, KC, 128], h_dt, tag="hT")
                for k in range(KC):
                    for j in range(T_PER_M):
                        t = m * T_PER_M + j
                        nc.scalar.activation(
                            out=hT[:, k, j * U : (j + 1) * U],
                            in_=pred_s[:, k, :],
                            func=mybir.ActivationFunctionType.Tanh,
                            bias=enc_s[:, k, t : t + 1],
                        )
                for n in range(NB):
                    ps = psum_pool.tile([128, 512], mybir.dt.float32, tag="ps", name="ps")
                    for k in range(KC):
                        nc.tensor.matmul(
                            ps,
                            hT[:, k, :],
                            w_s[k, n],
                            start=(k == 0),
                            stop=(k == KC - 1),
                        )
                    ob = opool.tile([128, 512], f32, tag="ob")
                    nc.vector.tensor_add(
                        ob, ps, bias_s[:, n * 512 : (n + 1) * 512]
                    )
                    nc.sync.dma_start(
                        out=out[m * 128 : (m + 1) * 128, n * 512 : (n + 1) * 512],
                        in_=ob,
                    )

    nc.compile()
    return nc


_NC_CACHE = {}


def _get_nc(mode: str):
    if mode not in _NC_CACHE:
        _NC_CACHE[mode] = build_nc(mode)
    return _NC_CACHE[mode]


def kernel(enc_out, pred_out, W_out, b_out, _trace=False):
    if not _trace:
        # the axon trace path needs antenv.axon_hooks, absent here
        os.environ["BASS_NEVER_TRACE"] = "1"
    enc_out = np.asarray(enc_out, dtype=np.float32)
    pred_out = np.asarray(pred_out, dtype=np.float32)
    W_out = np.asarray(W_out, dtype=np.float32)
    b_out = np.asarray(b_out, dtype=np.float32)

    mode = MM_MODE
    nc = _get_nc(mode)

    wT = np.ascontiguousarray(W_out.T)  # [D, V]
    if mode == "bf16":
        import ml_dtypes

        wT = wT.astype(ml_dtypes.bfloat16)
    elif mode == "fp16":
        wT = wT.astype(np.float16)
    bias2d = np.ascontiguousarray(b_out.reshape(1, V))

    in_maps = []
    for i in range(N_CORES):
        b_idx = i // (N_CORES // B)
        t0 = (i % (N_CORES // B)) * T_PER_CORE
        in_maps.append(
            {
                "encT": np.ascontiguousarray(enc_out[b_idx, t0 : t0 + T_PER_CORE].T),
                "predT": np.ascontiguousarray(pred_out[b_idx].T),
                "wT": wT,
                "bias": bias2d,
            }
        )

    res = run_bass_kernel_spmd(
        nc, in_maps, core_ids=list(range(N_CORES)), trace=_trace
    )

    out = np.empty((B, T, U, V), dtype=np.float32)
    for i in range(N_CORES):
        b_idx = i // (N_CORES // B)
        t0 = (i % (N_CORES // B)) * T_PER_CORE
        out[b_idx, t0 : t0 + T_PER_CORE] = res.results[i]["out"].reshape(
            T_PER_CORE, U, V
        )
    if _trace:
        return out, res
    return out


# revision 4
# speedup vs baseline: 1.0071x; 1.0071x over previous
"""RNN-T Joint network kernel for Trainium2 (Bass/Tile), 8-core SPMD.

Problem: out[b,t,u,v] = tanh(enc[b,t,:] + pred[b,u,:]) @ W[v,:] + bias[v]
  B=4, T=256, U=64, D=640, V=4096  (fp32 in/out)

Sharding: data-parallel over (B,T). Core i handles b = i//2, t in
[(i%2)*128, (i%2)*128+128). Each core computes an [128*64, 4096] slice of
the output; no collectives needed.

Device kernel (per core):
  - host pre-permutes operands so the contraction dim D sits on SBUF
    partitions AND every DMA line is contiguous DRAM: encP/predP are
    [128p, KC*len] row-major, W is packed per (k,n) block [128, 512].
  - hT[d, (t,u)] = tanh(predT[d,u] + encT[d,t]) via scalar-engine
    activation with per-partition bias (one instr per (d-chunk, t)).
  - PE matmul per (m-chunk, psum-bank): psum[128m, 512v] +=
    hT[k][:,m].T @ w[k,n] over 5 k-chunks; operands in fp16 so the PE
    streams 1 row/cycle and LDWEIGHTS overlaps the previous matmul
    (f32r serializes the weight load: 272 ns vs 216 ns per matmul).
  - per-bank epilogue: DVE adds bias PSUM->SBUF, then a 256KB DMA out.
    n-outer/k-inner keeps drains and output DMA evenly spread, which
    shrinks the kernel tail to one bank's worth of work.
  - enc/pred/bias ride the vector engine's DMA queue while the 4MB
    weight stream (issued in first-consumption (n,k) order) and the
    output stream share the sync queue, so the first tanh/matmul isn't
    gated on the weight preload.
"""

import os
import sys

import numpy as np

if "/root/.axon_site/_ro/trn_rl_repo" not in sys.path:
    sys.path.append("/root/.axon_site/_ro/trn_rl_repo")

import concourse.bass as bass  # noqa: E402
import concourse.mybir as mybir  # noqa: E402
import concourse.tile as tile  # noqa: E402
from concourse import bacc  # noqa: E402
from concourse.bass_utils import run_bass_kernel_spmd  # noqa: E402

B, T, U, D, V = 4, 256, 64, 640, 4096
N_CORES = 8
T_PER_CORE = T // (N_CORES // B)  # 128
ROWS = T_PER_CORE * U  # 8192 rows per core
KC = D // 128  # 5 k-chunks
NB = V // 512  # 8 psum banks per row-chunk
M_CHUNKS = ROWS // 128  # 64  (each = 2 t values x 64 u)
T_PER_M = 128 // U  # 2

# matmul dtype mode: "fp16"/"bf16" (1 cyc/row, hidden weight loads),
# "f32r" (1 cyc/row but serialized ldweights), "f32" (exact, 4 cyc/row)
MM_MODE = os.environ.get("JOINT_MM_MODE", "fp16")


def build_nc(mode: str):
    nc = bacc.Bacc("TRN2", target_bir_lowering=False, debug=False)

    f32 = mybir.dt.float32
    w_dt = {
        "bf16": mybir.dt.bfloat16,
        "fp16": mybir.dt.float16,
        "f32r": mybir.dt.float32r,
    }.get(mode, f32)

    encP_d = nc.dram_tensor("encP", [128, KC * T_PER_CORE], f32, kind="ExternalInput")
    predP_d = nc.dram_tensor("predP", [128, KC * U], f32, kind="ExternalInput")
    wP_d = nc.dram_tensor("wP", [KC * NB * 128, 512], w_dt, kind="ExternalInput")
    bias_d = nc.dram_tensor("bias", [1, V], f32, kind="ExternalInput")
    out_d = nc.dram_tensor("out", [ROWS, V], f32, kind="ExternalOutput")

    encP = encP_d.ap().rearrange("p (k t) -> p k t", k=KC)
    predP = predP_d.ap().rearrange("p (k u) -> p k u", k=KC)
    wP = wP_d.ap().rearrange("(k n p) c -> k n p c", k=KC, n=NB)
    out = out_d.ap()

    h_dt = w_dt

    with tile.TileContext(nc) as tc:
        with (
            tc.tile_pool(name="singles", bufs=1) as singles,
            tc.tile_pool(name="hpool", bufs=3) as hpool,
            tc.tile_pool(name="opool", bufs=12) as opool,
            tc.tile_pool(name="psum", bufs=8, space="PSUM") as psum_pool,
        ):
            # enc/pred/bias on the scalar engine's DMA queue: small, needed
            # first, and must not queue behind the 4MB weight stream.
            enc_s = singles.tile([128, KC, T_PER_CORE], f32, tag="enc")
            nc.scalar.dma_start(out=enc_s, in_=encP)
            pred_s = singles.tile([128, KC, U], f32, tag="pred")
            nc.scalar.dma_start(out=pred_s, in_=predP)
            bias_s = singles.tile([128, V], f32, tag="bias")
            nc.scalar.dma_start(out=bias_s, in_=bias_d.ap().to_broadcast((128, V)))
            # weights on the sync queue in first-consumption (n,k) order;
            # each block is a contiguous 128KB read (16KB packets).
            w_s = {}
            for n in range(NB):
                for k in range(KC):
                    w_s[k, n] = singles.tile(
                        [128, 512], w_dt, tag=f"w{k}_{n}", name=f"w{k}_{n}"
                    )
                    nc.sync.dma_start(out=w_s[k, n], in_=wP[k, n])

            for m in range(M_CHUNKS):
                hT = hpool.tile([128, KC, 128], h_dt, tag="hT")
                for k in range(KC):
                    for j in range(T_PER_M):
                        t = m * T_PER_M + j
                        nc.scalar.activation(
                            out=hT[:, k, j * U : (j + 1) * U],
                            in_=pred_s[:, k, :],
                            func=mybir.ActivationFunctionType.Tanh,
                            bias=enc_s[:, k, t : t + 1],
                        )
                for n in range(NB):
                    ps = psum_pool.tile([128, 512], mybir.dt.float32, tag="ps", name="ps")
                    for k in range(KC):
                        nc.tensor.matmul(
                            ps,
                            hT[:, k, :],
                            w_s[k, n],
                            start=(k == 0),
                            stop=(k == KC - 1),
                        )
                    ob = opool.tile([128, 512], f32, tag="ob")
                    nc.vector.tensor_add(
                        ob, ps, bias_s[:, n * 512 : (n + 1) * 512]
                    )
                    nc.sync.dma_start(
                        out=out[m * 128 : (m + 1) * 128, n * 512 : (n + 1) * 512],
                        in_=ob,
                    )

    nc.compile()
    return nc


_NC_CACHE = {}


def _get_nc(mode: str):
    if mode not in _NC_CACHE:
        _NC_CACHE[mode] = build_nc(mode)
    return _NC_CACHE[mode]


def _pack_dk(x):
    """[len, D] -> [128p, KC*len] with row d = k*128 + p."""
    ln = x.shape[0]
    return np.ascontiguousarray(
        x.T.reshape(KC, 128, ln).transpose(1, 0, 2).reshape(128, KC * ln)
    )


def kernel(enc_out, pred_out, W_out, b_out, _trace=False):
    if not _trace:
        # the axon trace path needs antenv.axon_hooks, absent here
        os.environ["BASS_NEVER_TRACE"] = "1"
    enc_out = np.asarray(enc_out, dtype=np.float32)
    pred_out = np.asarray(pred_out, dtype=np.float32)
    W_out = np.asarray(W_out, dtype=np.float32)
    b_out = np.asarray(b_out, dtype=np.float32)

    mode = MM_MODE
    nc = _get_nc(mode)

    wT = W_out.T  # [D, V]
    if mode == "bf16":
        import ml_dtypes

        wT = wT.astype(ml_dtypes.bfloat16)
    elif mode == "fp16":
        wT = wT.astype(np.float16)
    # pack into per-(k,n) contiguous [128, 512] blocks, (k, n) block order
    wP = np.ascontiguousarray(
        wT.reshape(KC, 128, NB, 512).transpose(0, 2, 1, 3).reshape(KC * NB * 128, 512)
    )
    bias2d = np.ascontiguousarray(b_out.reshape(1, V))

    in_maps = []
    for i in range(N_CORES):
        b_idx = i // (N_CORES // B)
        t0 = (i % (N_CORES // B)) * T_PER_CORE
        in_maps.append(
            {
                "encP": _pack_dk(enc_out[b_idx, t0 : t0 + T_PER_CORE]),
                "predP": _pack_dk(pred_out[b_idx]),
                "wP": wP,
                "bias": bias2d,
            }
        )

    res = run_bass_kernel_spmd(
        nc, in_maps, core_ids=list(range(N_CORES)), trace=_trace
    )

    out = np.empty((B, T, U, V), dtype=np.float32)
    for i in range(N_CORES):
        b_idx = i // (N_CORES // B)
        t0 = (i % (N_CORES // B)) * T_PER_CORE
        out[b_idx, t0 : t0 + T_PER_CORE] = res.results[i]["out"].reshape(
            T_PER_CORE, U, V
        )
    if _trace:
        return out, res
    return out


# revision 7
# speedup vs baseline: 1.0232x; 1.0160x over previous
"""RNN-T Joint network kernel for Trainium2 (Bass/Tile), 8-core SPMD.

Problem: out[b,t,u,v] = tanh(enc[b,t,:] + pred[b,u,:]) @ W[v,:] + bias[v]
  B=4, T=256, U=64, D=640, V=4096  (fp32 in/out)

Sharding: data-parallel over (B,T). Core i handles b = i//2, t in
[(i%2)*128, (i%2)*128+128). Each core computes an [128*64, 4096] slice of
the output; no collectives needed.

Device kernel (per core):
  - host pre-permutes operands so the contraction dim D sits on SBUF
    partitions AND every DMA line is contiguous DRAM: encP/predP are
    [128p, KC*len] row-major, W is packed per (k,n) block [128, 512].
  - hT[d, (t,u)] = tanh(predT[d,u] + encT[d,t]) via scalar-engine
    activation with per-partition bias (one instr per (d-chunk, t)).
  - PE matmul per (m-chunk, psum-bank): psum[128m, 512v] +=
    hT[k][:,m].T @ w[k,n] over 5 k-chunks; operands in fp16 so the PE
    streams 1 row/cycle and LDWEIGHTS overlaps the previous matmul
    (f32r serializes the weight load: 272 ns vs 216 ns per matmul).
  - per-bank epilogue: DVE adds bias PSUM->SBUF, then a 256KB DMA out.
    n-outer/k-inner keeps drains and output DMA evenly spread, which
    shrinks the kernel tail to one bank's worth of work.
  - enc/pred/bias ride the vector engine's DMA queue while the 4MB
    weight stream (issued in first-consumption (n,k) order) and the
    output stream share the sync queue, so the first tanh/matmul isn't
    gated on the weight preload.
"""

import os
import sys

import numpy as np

if "/root/.axon_site/_ro/trn_rl_repo" not in sys.path:
    sys.path.append("/root/.axon_site/_ro/trn_rl_repo")

import concourse.bass as bass  # noqa: E402
import concourse.mybir as mybir  # noqa: E402
import concourse.tile as tile  # noqa: E402
from concourse import bacc  # noqa: E402
from concourse.bass_utils import run_bass_kernel_spmd  # noqa: E402

B, T, U, D, V = 4, 256, 64, 640, 4096
N_CORES = 8
T_PER_CORE = T // (N_CORES // B)  # 128
ROWS = T_PER_CORE * U  # 8192 rows per core
KC = D // 128  # 5 k-chunks
NB = V // 512  # 8 psum banks per row-chunk
M_CHUNKS = ROWS // 128  # 64  (each = 2 t values x 64 u)
T_PER_M = 128 // U  # 2

# matmul dtype mode: "fp16"/"bf16" (1 cyc/row, hidden weight loads),
# "f32r" (1 cyc/row but serialized ldweights), "f32" (exact, 4 cyc/row)
MM_MODE = os.environ.get("JOINT_MM_MODE", "fp16")


def build_nc(mode: str):
    nc = bacc.Bacc("TRN2", target_bir_lowering=False, debug=False)

    f32 = mybir.dt.float32
    w_dt = {
        "bf16": mybir.dt.bfloat16,
        "fp16": mybir.dt.float16,
        "f32r": mybir.dt.float32r,
    }.get(mode, f32)

    encP_d = nc.dram_tensor("encP", [128, KC * T_PER_CORE], f32, kind="ExternalInput")
    predP_d = nc.dram_tensor("predP", [128, KC * U], f32, kind="ExternalInput")
    wP_d = nc.dram_tensor("wP", [128, NB * KC * 512], w_dt, kind="ExternalInput")
    bias_d = nc.dram_tensor("bias", [1, V], f32, kind="ExternalInput")
    out_d = nc.dram_tensor("out", [ROWS, V], f32, kind="ExternalOutput")

    encP = encP_d.ap().rearrange("p (k t) -> p k t", k=KC)
    predP = predP_d.ap().rearrange("p (k u) -> p k u", k=KC)
    wP = wP_d.ap().rearrange("p (n k c) -> p n k c", n=NB, k=KC)
    out = out_d.ap()

    h_dt = w_dt

    with tile.TileContext(nc) as tc:
        with (
            tc.tile_pool(name="singles", bufs=1) as singles,
            tc.tile_pool(name="hpool", bufs=3) as hpool,
            tc.tile_pool(name="opool", bufs=12) as opool,
            tc.tile_pool(name="psum", bufs=8, space="PSUM") as psum_pool,
        ):
            # enc/pred/bias on the scalar engine's DMA queue: small, needed
            # first, and must not queue behind the 4MB weight stream.
            enc_s = singles.tile([128, KC, T_PER_CORE], f32, tag="enc")
            nc.scalar.dma_start(out=enc_s, in_=encP)
            pred_s = singles.tile([128, KC, U], f32, tag="pred")
            nc.scalar.dma_start(out=pred_s, in_=predP)
            bias_s = singles.tile([128, V], f32, tag="bias")
            nc.scalar.dma_start(out=bias_s, in_=bias_d.ap().to_broadcast((128, V)))
            # weights: DRAM is partition-major (40KB contiguous/partition in
            # bank-then-k order) so read packets hit the 16KB max. Four
            # column-range DMAs stage banks in consumption order: read-DMA
            # packets are one per SBUF partition line, so bank 0 alone
            # (5KB lines) flows first while the wide tail DMAs stream in.
            w_all = singles.tile([128, NB, KC, 512], w_dt, tag="w")
            for lo, hi in ((0, 1), (1, 3), (3, 5), (5, 8)):
                nc.sync.dma_start(
                    out=w_all[:, lo:hi], in_=wP[:, lo:hi]
                )
            w_s = {(k, n): w_all[:, n, k, :] for n in range(NB) for k in range(KC)}

            for m in range(M_CHUNKS):
                hT = hpool.tile([128, KC, 128], h_dt, tag="hT")
                for k in range(KC):
                    for j in range(T_PER_M):
                        t = m * T_PER_M + j
                        nc.scalar.activation(
                            out=hT[:, k, j * U : (j + 1) * U],
                            in_=pred_s[:, k, :],
                            func=mybir.ActivationFunctionType.Tanh,
                            bias=enc_s[:, k, t : t + 1],
                        )
                for n in range(NB):
                    ps = psum_pool.tile([128, 512], mybir.dt.float32, tag="ps", name="ps")
                    for k in range(KC):
                        nc.tensor.matmul(
                            ps,
                            hT[:, k, :],
                            w_s[k, n],
                            start=(k == 0),
                            stop=(k == KC - 1),
                        )
                    ob = opool.tile([128, 512], f32, tag="ob")
                    nc.vector.tensor_add(
                        ob, ps, bias_s[:, n * 512 : (n + 1) * 512]
                    )
                    nc.sync.dma_start(
                        out=out[m * 128 : (m + 1) * 128, n * 512 : (n + 1) * 512],
                        in_=ob,
                    )

    nc.compile()
    return nc


_NC_CACHE = {}


def _get_nc(mode: str):
    if mode not in _NC_CACHE:
        _NC_CACHE[mode] = build_nc(mode)
    return _NC_CACHE[mode]


def _pack_dk(x):
    """[len, D] -> [128p, KC*len] with row d = k*128 + p."""
    ln = x.shape[0]
    return np.ascontiguousarray(
        x.T.reshape(KC, 128, ln).transpose(1, 0, 2).reshape(128, KC * ln)
    )


def kernel(enc_out, pred_out, W_out, b_out, _trace=False):
    if not _trace:
        # the axon trace path needs antenv.axon_hooks, absent here
        os.environ["BASS_NEVER_TRACE"] = "1"
    enc_out = np.asarray(enc_out, dtype=np.float32)
    pred_out = np.asarray(pred_out, dtype=np.float32)
    W_out = np.asarray(W_out, dtype=np.float32)
    b_out = np.asarray(b_out, dtype=np.float32)

    mode = MM_MODE
    nc = _get_nc(mode)

    wT = W_out.T  # [D, V]
    if mode == "bf16":
        import ml_dtypes

        wT = wT.astype(ml_dtypes.bfloat16)
    elif mode == "fp16":
        wT = wT.astype(np.float16)
    # partition-major pack: wP[p, n, k, c] = wT[k*128+p, n*512+c]
    wP = np.ascontiguousarray(
        wT.reshape(KC, 128, NB, 512).transpose(1, 2, 0, 3).reshape(128, NB * KC * 512)
    )
    bias2d = np.ascontiguousarray(b_out.reshape(1, V))

    in_maps = []
    for i in range(N_CORES):
        b_idx = i // (N_CORES // B)
        t0 = (i % (N_CORES // B)) * T_PER_CORE
        in_maps.append(
            {
                "encP": _pack_dk(enc_out[b_idx, t0 : t0 + T_PER_CORE]),
                "predP": _pack_dk(pred_out[b_idx]),
                "wP": wP,
                "bias": bias2d,
            }
        )

    res = run_bass_kernel_spmd(
        nc, in_maps, core_ids=list(range(N_CORES)), trace=_trace
    )

    out = np.empty((B, T, U, V), dtype=np.float32)
    for i in range(N_CORES):
        b_idx = i // (N_CORES // B)
        t0 = (i % (N_CORES // B)) * T_PER_CORE
        out[b_idx, t0 : t0 + T_PER_CORE] = res.results[i]["out"].reshape(
            T_PER_CORE, U, V
        )
    if _trace:
        return out, res
    return out


# revision 10
# speedup vs baseline: 1.0273x; 1.0040x over previous
"""RNN-T Joint network kernel for Trainium2 (Bass/Tile), 8-core SPMD.

Problem: out[b,t,u,v] = tanh(enc[b,t,:] + pred[b,u,:]) @ W[v,:] + bias[v]
  B=4, T=256, U=64, D=640, V=4096  (fp32 in/out)

Sharding: data-parallel over (B,T). Core i handles b = i//2, t in
[(i%2)*128, (i%2)*128+128). Each core computes an [128*64, 4096] slice of
the output; no collectives needed.

Device kernel (per core):
  - host pre-permutes operands so the contraction dim D sits on SBUF
    partitions AND every DMA line is contiguous DRAM: encP/predP are
    [128p, KC*len] row-major, W is packed per (k,n) block [128, 512].
  - hT[d, (t,u)] = tanh(predT[d,u] + encT[d,t]) via scalar-engine
    activation with per-partition bias (one instr per (d-chunk, t)).
  - PE matmul per (m-chunk, psum-bank): psum[128m, 512v] +=
    hT[k][:,m].T @ w[k,n] over 5 k-chunks; operands in fp16 so the PE
    streams 1 row/cycle and LDWEIGHTS overlaps the previous matmul
    (f32r serializes the weight load: 272 ns vs 216 ns per matmul).
  - per-bank epilogue: DVE adds bias PSUM->SBUF, then a 256KB DMA out.
    n-outer/k-inner keeps drains and output DMA evenly spread, which
    shrinks the kernel tail to one bank's worth of work.
  - enc/pred/bias ride the vector engine's DMA queue while the 4MB
    weight stream (issued in first-consumption (n,k) order) and the
    output stream share the sync queue, so the first tanh/matmul isn't
    gated on the weight preload.
"""

import os
import sys

import numpy as np

if "/root/.axon_site/_ro/trn_rl_repo" not in sys.path:
    sys.path.append("/root/.axon_site/_ro/trn_rl_repo")

import concourse.bass as bass  # noqa: E402
import concourse.mybir as mybir  # noqa: E402
import concourse.tile as tile  # noqa: E402
from concourse import bacc  # noqa: E402
from concourse.bass_utils import run_bass_kernel_spmd  # noqa: E402

B, T, U, D, V = 4, 256, 64, 640, 4096
N_CORES = 8
T_PER_CORE = T // (N_CORES // B)  # 128
ROWS = T_PER_CORE * U  # 8192 rows per core
KC = D // 128  # 5 k-chunks
NB = V // 512  # 8 psum banks per row-chunk
M_CHUNKS = ROWS // 128  # 64  (each = 2 t values x 64 u)
T_PER_M = 128 // U  # 2

# matmul dtype mode: "fp16"/"bf16" (1 cyc/row, hidden weight loads),
# "f32r" (1 cyc/row but serialized ldweights), "f32" (exact, 4 cyc/row)
MM_MODE = os.environ.get("JOINT_MM_MODE", "fp16")


def build_nc(mode: str):
    nc = bacc.Bacc("TRN2", target_bir_lowering=False, debug=False)

    f32 = mybir.dt.float32
    w_dt = {
        "bf16": mybir.dt.bfloat16,
        "fp16": mybir.dt.float16,
        "f32r": mybir.dt.float32r,
    }.get(mode, f32)

    encP_d = nc.dram_tensor("encP", [128, KC * T_PER_CORE], f32, kind="ExternalInput")
    predP_d = nc.dram_tensor("predP", [128, KC * U], f32, kind="ExternalInput")
    wP_d = nc.dram_tensor("wP", [128, NB * KC * 512], w_dt, kind="ExternalInput")
    bias_d = nc.dram_tensor("bias", [1, V], f32, kind="ExternalInput")
    out_d = nc.dram_tensor("out", [ROWS, V], f32, kind="ExternalOutput")

    encP = encP_d.ap().rearrange("p (k t) -> p k t", k=KC)
    predP = predP_d.ap().rearrange("p (k u) -> p k u", k=KC)
    wP = wP_d.ap().rearrange("p (n k c) -> p n k c", n=NB, k=KC)
    out = out_d.ap()

    h_dt = w_dt

    with tile.TileContext(nc) as tc:
        with (
            tc.tile_pool(name="singles", bufs=1) as singles,
            tc.tile_pool(name="hpool", bufs=3) as hpool,
            tc.tile_pool(name="opool", bufs=12) as opool,
            tc.tile_pool(name="psum", bufs=8, space="PSUM") as psum_pool,
        ):
            # enc/pred at the head of the sync queue (first consumers);
            # bias rides the scalar engine's queue in parallel.
            enc_s = singles.tile([128, KC, T_PER_CORE], f32, tag="enc")
            nc.sync.dma_start(out=enc_s, in_=encP)
            pred_s = singles.tile([128, KC, U], f32, tag="pred")
            nc.sync.dma_start(out=pred_s, in_=predP)
            bias_s = singles.tile([128, V], f32, tag="bias")
            nc.scalar.dma_start(out=bias_s, in_=bias_d.ap().to_broadcast((128, V)))
            # weights: DRAM is partition-major (40KB contiguous/partition in
            # bank-then-k order) so read packets aggregate well (read-DMA
            # packets are one per SBUF partition line). Column-range DMAs
            # stage banks in consumption order.
            w_all = singles.tile([128, NB, KC, 512], w_dt, tag="w")
            for lo, hi in ((0, 1), (1, 2), (2, 4), (4, 6), (6, 8)):
                nc.sync.dma_start(
                    out=w_all[:, lo:hi], in_=wP[:, lo:hi]
                )

            for m in range(M_CHUNKS):
                hT = hpool.tile([128, KC, 128], h_dt, tag="hT")
                for k in range(KC):
                    for j in range(T_PER_M):
                        t = m * T_PER_M + j
                        nc.scalar.activation(
                            out=hT[:, k, j * U : (j + 1) * U],
                            in_=pred_s[:, k, :],
                            func=mybir.ActivationFunctionType.Tanh,
                            bias=enc_s[:, k, t : t + 1],
                        )
                for n in range(NB):
                    ps = psum_pool.tile([128, 512], mybir.dt.float32, tag="ps", name="ps")
                    for k in range(KC):
                        nc.tensor.matmul(
                            ps,
                            hT[:, k, :],
                            w_all[:, n, k, :],
                            start=(k == 0),
                            stop=(k == KC - 1),
                        )
                    ob = opool.tile([128, 512], f32, tag="ob")
                    nc.vector.tensor_add(
                        ob, ps, bias_s[:, n * 512 : (n + 1) * 512]
                    )
                    nc.sync.dma_start(
                        out=out[m * 128 : (m + 1) * 128, n * 512 : (n + 1) * 512],
                        in_=ob,
                    )

    nc.compile()
    return nc


_NC_CACHE = {}


def _get_nc(mode: str):
    if mode not in _NC_CACHE:
        _NC_CACHE[mode] = build_nc(mode)
    return _NC_CACHE[mode]


def _pack_dk(x):
    """[len, D] -> [128p, KC*len] with row d = k*128 + p."""
    ln = x.shape[0]
    return np.ascontiguousarray(
        x.T.reshape(KC, 128, ln).transpose(1, 0, 2).reshape(128, KC * ln)
    )


def kernel(enc_out, pred_out, W_out, b_out, _trace=False):
    if not _trace:
        # the axon trace path needs antenv.axon_hooks, absent here
        os.environ["BASS_NEVER_TRACE"] = "1"
    enc_out = np.asarray(enc_out, dtype=np.float32)
    pred_out = np.asarray(pred_out, dtype=np.float32)
    W_out = np.asarray(W_out, dtype=np.float32)
    b_out = np.asarray(b_out, dtype=np.float32)

    mode = MM_MODE
    nc = _get_nc(mode)

    wT = W_out.T  # [D, V]
    if mode == "bf16":
        import ml_dtypes

        wT = wT.astype(ml_dtypes.bfloat16)
    elif mode == "fp16":
        wT = wT.astype(np.float16)
    # partition-major pack: wP[p, n, k, c] = wT[k*128+p, n*512+c]
    wP = np.ascontiguousarray(
        wT.reshape(KC, 128, NB, 512).transpose(1, 2, 0, 3).reshape(128, NB * KC * 512)
    )
    bias2d = np.ascontiguousarray(b_out.reshape(1, V))

    in_maps = []
    for i in range(N_CORES):
        b_idx = i // (N_CORES // B)
        t0 = (i % (N_CORES // B)) * T_PER_CORE
        in_maps.append(
            {
                "encP": _pack_dk(enc_out[b_idx, t0 : t0 + T_PER_CORE]),
                "predP": _pack_dk(pred_out[b_idx]),
                "wP": wP,
                "bias": bias2d,
            }
        )

    res = run_bass_kernel_spmd(
        nc, in_maps, core_ids=list(range(N_CORES)), trace=_trace
    )

    out = np.empty((B, T, U, V), dtype=np.float32)
    for i in range(N_CORES):
        b_idx = i // (N_CORES // B)
        t0 = (i % (N_CORES // B)) * T_PER_CORE
        out[b_idx, t0 : t0 + T_PER_CORE] = res.results[i]["out"].reshape(
            T_PER_CORE, U, V
        )
    if _trace:
        return out, res
    return out
